# revision 4
# baseline (speedup 1.0000x reference)
import numpy as np
import ml_dtypes

BF = ml_dtypes.bfloat16
B, N, WT, F, H = 64, 512, 24, 16, 128
NL = N // 8   # nodes per core
NT = WT // 2  # timestep pairs
KC = 49       # fused input contraction: 3 hops * 16 f + ones row

# xstack partition layout (rows of the K=49 contraction):
#   p 0-15  : hop0 (raw x features)
#   p 16    : ones (carries all input-side biases)
#   p 17-32 : hop1 (A @ x features)
#   p 33-48 : hop2 (A^2 @ x features)


def _build(nc, bass, mybir, tile):
    f32 = mybir.dt.float32
    bf16 = mybir.dt.bfloat16
    Alu = mybir.AluOpType
    Act = mybir.ActivationFunctionType

    t_a1t = nc.dram_tensor("a1t", [128, 4, NL], bf16, kind="ExternalInput").ap()
    t_a2t = nc.dram_tensor("a2t", [128, 4, NL], bf16, kind="ExternalInput").ap()
    t_xm = nc.dram_tensor("xm", [B, 128, 4, WT * F], bf16, kind="ExternalInput").ap()
    # hop0 features + ones row, per timestep pair: [pair, 17, nl, 2, b]
    t_x0b = nc.dram_tensor("x0b", [NT, 17, NL, 2, B], bf16, kind="ExternalInput").ap()
    t_wc = nc.dram_tensor("wc", [KC, NL, 3, 128], bf16, kind="ExternalInput").ap()
    t_whh = nc.dram_tensor("whh", [128, NL, 3, 128], bf16, kind="ExternalInput").ap()
    t_bhn = nc.dram_tensor("bhn", [8, 8, 128], bf16, kind="ExternalInput").ap()
    t_ind = nc.dram_tensor("ind", [8, 8, B], bf16, kind="ExternalInput").ap()
    t_wout = nc.dram_tensor("wout", [128, F], bf16, kind="ExternalInput").ap()
    t_bout = nc.dram_tensor("bout", [128, F], f32, kind="ExternalInput").ap()
    t_out = nc.dram_tensor("out", [128, 32, F], f32, kind="ExternalOutput").ap()

    with tile.TileContext(nc) as tc:
        with (
            tc.tile_pool(name="const", bufs=1) as cpool,
            tc.tile_pool(name="hpool", bufs=1) as hpool,
            tc.tile_pool(name="stage", bufs=1) as spool,
        ):
            a1t = cpool.tile([128, 4, NL], bf16)
            a2t = cpool.tile([128, 4, NL], bf16)
            wc = cpool.tile([KC, NL, 3, 128], bf16)
            whh = cpool.tile([128, NL, 3, 128], bf16)
            bhn = cpool.tile([8, 8, 128], bf16)
            ind = cpool.tile([8, 8, B], bf16)
            wout = cpool.tile([128, F], bf16)
            bout = cpool.tile([128, F], f32)
            for sb, dr in [(a1t, t_a1t), (a2t, t_a2t), (wc, t_wc), (whh, t_whh),
                           (bhn, t_bhn), (ind, t_ind), (wout, t_wout),
                           (bout, t_bout)]:
                nc.sync.dma_start(sb[:], dr[:])
            h = hpool.tile([128, NL, B], bf16)
            nc.any.memset(h[:], 0.0)

            # staging for diffusion hops, baseline layout [(w%8)*16+f, cc, nl, b]
            xc1 = spool.tile([128, 3, NL, B], bf16)
            xc2 = spool.tile([128, 3, NL, B], bf16)

            # ---- phase 1: diffusion hops (contract over source nodes) ----
            with (
                tc.tile_pool(name="xmb", bufs=2) as xmpool,
                tc.tile_pool(name="p1", bufs=2, space="PSUM") as p1pool,
            ):
                for b in range(B):
                    xmb = xmpool.tile([128, 4, WT * F], bf16)
                    nc.sync.dma_start(xmb[:], t_xm[b])
                    P1 = p1pool.tile([128, 3, NL], f32, tag="P1")
                    P2 = p1pool.tile([128, 3, NL], f32, tag="P2")
                    for mc in range(4):
                        for cc in range(3):
                            lhsT = xmb[:, mc, 128 * cc:128 * cc + 128]
                            nc.tensor.matmul(P1[:, cc, :], lhsT, a1t[:, mc, :],
                                             start=(mc == 0), stop=(mc == 3))
                            nc.tensor.matmul(P2[:, cc, :], lhsT, a2t[:, mc, :],
                                             start=(mc == 0), stop=(mc == 3))
                    nc.vector.tensor_copy(xc1[:, :, :, b], P1[:])
                    nc.scalar.copy(xc2[:, :, :, b], P2[:])

            # ---- phase 2: GRU over time, input projection fused into Wc ----
            with (
                tc.tile_pool(name="xs", bufs=2) as xspool,
                tc.tile_pool(name="gp", bufs=2, space="PSUM") as gppool,
                tc.tile_pool(name="hn", bufs=2, space="PSUM") as hnpool,
                tc.tile_pool(name="ew", bufs=2) as ewpool,
            ):
                for t in range(NT):
                    xs = xspool.tile([128, NL, 2, B], bf16, tag="xs")
                    # hop0 rows 0-15 + ones row 16 straight from DRAM
                    nc.sync.dma_start(xs[0:17, :, :, :], t_x0b[t])
                    # hop1/hop2 rows via SBUF->SBUF partition restructure
                    for wsub in range(2):
                        w = 2 * t + wsub
                        wo, ccw = w % 8, w // 8
                        src1 = xc1[16 * wo:16 * wo + 16, ccw, :, :]
                        src2 = xc2[16 * wo:16 * wo + 16, ccw, :, :]
                        nc.sync.dma_start(xs[17:33, :, wsub, :], src1)
                        nc.sync.dma_start(xs[33:49, :, wsub, :], src2)

                    for wsub in range(2):
                        for g in range(8):
                            ns = slice(8 * g, 8 * g + 8)
                            P = gppool.tile([128, 8, 3, B], f32, tag="P")
                            Phn = hnpool.tile([128, 8, B], f32, tag="Phn")
                            # b_hn via indicator matmul (opens accumulation)
                            nc.tensor.matmul(Phn[:], bhn[:, g, :], ind[:],
                                             start=True, stop=False,
                                             skip_group_check=True)
                            for j in range(8):
                                n = 8 * g + j
                                xsn = xs[0:KC, n, wsub, :]
                                hn_ = h[:, n, :]
                                for gc in range(2):
                                    o = P[:, j, gc, :]
                                    nc.tensor.matmul(o, wc[:, n, gc, :], xsn,
                                                     start=True, stop=False)
                                    nc.tensor.matmul(o, whh[:, n, gc, :], hn_,
                                                     start=False, stop=True)
                                nc.tensor.matmul(P[:, j, 2, :], wc[:, n, 2, :],
                                                 xsn, start=True, stop=True)
                                nc.tensor.matmul(Phn[:, j, :], whh[:, n, 2, :],
                                                 hn_, start=False,
                                                 stop=(j == 7),
                                                 skip_group_check=True)
                            # ---- gate elementwise ----
                            rz = ewpool.tile([128, 8, 2, B], bf16, tag="rz")
                            nc.scalar.activation(rz[:], P[:, :, 0:2, :],
                                                 Act.Sigmoid)
                            tt = ewpool.tile([128, 8, B], bf16, tag="tt")
                            nc.vector.tensor_tensor(tt[:], rz[:, :, 0, :],
                                                    Phn[:], Alu.mult)
                            ut = ewpool.tile([128, 8, B], bf16, tag="ut")
                            nc.vector.tensor_tensor(ut[:], tt[:], P[:, :, 2, :],
                                                    Alu.add)
                            nt = ewpool.tile([128, 8, B], bf16, tag="nt")
                            nc.scalar.activation(nt[:], ut[:], Act.Tanh)
                            st = ewpool.tile([128, 8, B], bf16, tag="st")
                            nc.gpsimd.tensor_tensor(st[:], h[:, ns, :], nt[:],
                                                    Alu.subtract)
                            vt = ewpool.tile([128, 8, B], bf16, tag="vt")
                            nc.vector.tensor_tensor(vt[:], rz[:, :, 1, :], st[:],
                                                    Alu.mult)
                            nc.vector.tensor_tensor(h[:, ns, :], nt[:], vt[:],
                                                    Alu.add)

            # ---- output projection ----
            with (
                tc.tile_pool(name="po", bufs=1, space="PSUM") as popool,
                tc.tile_pool(name="ou", bufs=1) as oupool,
            ):
                Po = popool.tile([128, 32, F], f32)
                for c in range(32):
                    nc.tensor.matmul(Po[:, c, :], h[:, 2 * c:2 * c + 2, :],
                                     wout[:], start=True, stop=True)
                outsb = oupool.tile([128, 32, F], f32)
                nc.vector.tensor_tensor(
                    outsb[:], Po[:], bout[:, None, :].to_broadcast((128, 32, F)),
                    Alu.add)
                nc.sync.dma_start(t_out[:], outsb[:])
    nc.compile()


def kernel(**inputs):
    import concourse.bacc as bacc
    import concourse.bass as bass
    import concourse.mybir as mybir
    import concourse.tile as tile
    from concourse import bass_utils

    x = np.asarray(inputs["x"], np.float32)
    A = np.asarray(inputs["A_fw"], np.float32)
    dcw = np.asarray(inputs["dc_weights"], np.float32)
    W_ih = np.asarray(inputs["W_ih"], np.float32)
    W_hh = np.asarray(inputs["W_hh"], np.float32)
    b_ih = np.asarray(inputs["b_ih"], np.float32)
    b_hh = np.asarray(inputs["b_hh"], np.float32)
    W_out = np.asarray(inputs["W_out"], np.float32)
    b_out = np.asarray(inputs["b_out"], np.float32)

    A2 = A @ A
    dc_all = np.stack([dcw[0:16], dcw[16:32] + dcw[32:48], dcw[48:64] + dcw[64:80]])
    xbf = x.astype(BF)
    xm = np.ascontiguousarray(xbf.reshape(B, 4, 128, WT * F).transpose(0, 2, 1, 3))
    wout_h = W_out.astype(BF)
    bout_h = np.tile(b_out[None, :], (128, 1)).astype(np.float32)

    # fused input weights: Wc[n, gate, j, (hop, f)] = sum_h W_ih[n, gj, h] dc[hop, f, h]
    Wg = W_ih.reshape(N, 3, 128, H)
    wc_full = np.einsum('ngjh,ofh->ngjof', Wg, dc_all)        # [N, 3, 128, 3, 16]
    bias_in = b_ih.reshape(N, 3, 128).copy()
    bias_in[:, 0:2, :] += b_hh.reshape(N, 3, 128)[:, 0:2, :]  # r,z combined bias
    # K-layout: rows 0-15 hop0, 16 ones(bias), 17-32 hop1, 33-48 hop2
    wc_k = np.zeros((N, 3, KC, 128), np.float32)
    wc_k[:, :, 0:16, :] = wc_full[:, :, :, 0, :].transpose(0, 1, 3, 2)
    wc_k[:, :, 16, :] = bias_in
    wc_k[:, :, 17:33, :] = wc_full[:, :, :, 1, :].transpose(0, 1, 3, 2)
    wc_k[:, :, 33:49, :] = wc_full[:, :, :, 2, :].transpose(0, 1, 3, 2)

    ind_h = np.zeros((8, 8, B), np.float32)
    for k in range(8):
        ind_h[k, k, :] = 1.0
    ind_h = ind_h.astype(BF)

    in_maps = []
    for c in range(8):
        ns = slice(c * NL, (c + 1) * NL)
        a1t = np.ascontiguousarray(
            A[ns].T.astype(BF).reshape(4, 128, NL).transpose(1, 0, 2))
        a2t = np.ascontiguousarray(
            A2[ns].T.astype(BF).reshape(4, 128, NL).transpose(1, 0, 2))
        xl = xbf[:, ns]  # [b, nl, w, f]
        # x0b[t, p, nl, wsub, b]: p 0-15 = f rows of x[w=2t+wsub], p16 = ones
        x0b = np.empty((NT, 17, NL, 2, B), np.float32)
        xw = np.asarray(xl, np.float32).transpose(2, 3, 1, 0)  # [w, f, nl, b]
        x0b[:, 0:16] = xw.reshape(NT, 2, 16, NL, B).transpose(0, 2, 3, 1, 4)
        x0b[:, 16] = 1.0
        wc_h = np.ascontiguousarray(
            wc_k[ns].transpose(2, 0, 1, 3)).astype(BF)       # [49, NL, 3, 128]
        whh_h = np.ascontiguousarray(
            W_hh[ns].transpose(2, 0, 1).astype(BF).reshape(128, NL, 3, 128))
        bhn_h = np.ascontiguousarray(
            b_hh[ns, 256:384].reshape(8, 8, 128)).astype(BF)  # [g, n8, j]
        bhn_h = np.ascontiguousarray(bhn_h.transpose(1, 0, 2))  # [n8, g, j]
        in_maps.append({
            "a1t": a1t, "a2t": a2t, "xm": xm,
            "x0b": np.ascontiguousarray(x0b).astype(BF),
            "wc": wc_h, "whh": whh_h, "bhn": bhn_h, "ind": ind_h,
            "wout": wout_h, "bout": bout_h,
        })

    nc = bacc.Bacc("TRN2", target_bir_lowering=False, debug=False, num_devices=8)
    _build(nc, bass, mybir, tile)
    import os, time
    trace = bool(os.environ.get("DGCN_TRACE"))
    res = bass_utils.run_bass_kernel_spmd(nc, in_maps, core_ids=list(range(8)),
                                          trace=trace)
    if trace and res.exec_time_ns:
        print(f"MEASURED exec_time_ns: {res.exec_time_ns}", flush=True)
        try:
            with open("/tmp/dgcn_exec_ns.txt", "w") as f:
                f.write(str(res.exec_time_ns))
        except Exception:
            pass
        if res.instructions_and_trace:
            print(f"trace: {res.instructions_and_trace[1]}", flush=True)
    if os.environ.get("DGCN_BENCH"):
        for it in range(int(os.environ["DGCN_BENCH"])):
            t0 = time.time()
            res = bass_utils.run_bass_kernel_spmd(nc, in_maps, core_ids=list(range(8)))
            print(f"bench iter {it}: {(time.time()-t0)*1e3:.1f} ms", flush=True)

    out = np.zeros((B, N, F), np.float32)
    for c in range(8):
        arr = res.results[c]["out"]  # [128, 32, F]
        tmp = arr.transpose(1, 0, 2).reshape(32, 2, B, F).transpose(2, 0, 1, 3)
        out[:, c * NL:(c + 1) * NL] = tmp.reshape(B, NL, F)
    return out


# revision 14
# speedup vs baseline: 1.8047x; 1.8047x over previous
import numpy as np
import ml_dtypes

BF = ml_dtypes.bfloat16
B, N, WT, F, H = 64, 512, 24, 16, 128
NL = N // 8   # nodes per core
NT = WT // 2  # timestep pairs
KC = 49       # fused input contraction: 3 hops * 16 f + ones row

# xstack partition layout (rows of the K=49 contraction):
#   p 0-15  : hop0 (raw x features)
#   p 16    : ones (carries all input-side biases)
#   p 17-32 : hop1 (A @ x features)
#   p 33-48 : hop2 (A^2 @ x features)


def _build(nc, bass, mybir, tile):
    f32 = mybir.dt.float32
    bf16 = mybir.dt.bfloat16
    Alu = mybir.AluOpType
    Act = mybir.ActivationFunctionType

    t_a1t = nc.dram_tensor("a1t", [128, 4, NL], bf16, kind="ExternalInput").ap()
    t_a2t = nc.dram_tensor("a2t", [128, 4, NL], bf16, kind="ExternalInput").ap()
    t_xm = nc.dram_tensor("xm", [B, 128, 4, WT * F], bf16, kind="ExternalInput").ap()
    # hop0 features + ones row, per timestep pair: [pair, 17, nl, 2, b]
    t_x0b = nc.dram_tensor("x0b", [NT, 17, NL, 2, B], bf16, kind="ExternalInput").ap()
    t_wc = nc.dram_tensor("wc", [128, NL, 3, 128], bf16, kind="ExternalInput").ap()
    t_whh = nc.dram_tensor("whh", [128, NL, 3, 128], bf16, kind="ExternalInput").ap()
    t_bhn = nc.dram_tensor("bhn", [128, 8, 128], bf16, kind="ExternalInput").ap()
    t_ind = nc.dram_tensor("ind", [128, 8, B], bf16, kind="ExternalInput").ap()
    t_zpad = nc.dram_tensor("zpad", [128 - KC, NL, 2, B], bf16,
                            kind="ExternalInput").ap()
    t_wout = nc.dram_tensor("wout", [128, F], bf16, kind="ExternalInput").ap()
    t_bout = nc.dram_tensor("bout", [128, F], f32, kind="ExternalInput").ap()
    t_out = nc.dram_tensor("out", [128, 32, F], f32, kind="ExternalOutput").ap()

    with tile.TileContext(nc) as tc:
        with (
            tc.tile_pool(name="const", bufs=1) as cpool,
            tc.tile_pool(name="hpool", bufs=1) as hpool,
            tc.tile_pool(name="stage", bufs=1) as spool,
        ):
            a1t = cpool.tile([128, 4, NL], bf16)
            a2t = cpool.tile([128, 4, NL], bf16)
            wc = cpool.tile([128, NL, 3, 128], bf16)
            whh = cpool.tile([128, NL, 3, 128], bf16)
            bhn = cpool.tile([128, 8, 128], bf16)
            ind = cpool.tile([128, 8, B], bf16)
            wout = cpool.tile([128, F], bf16)
            bout = cpool.tile([128, F], f32)
            for sb, dr in [(a1t, t_a1t), (a2t, t_a2t), (wc, t_wc), (whh, t_whh),
                           (bhn, t_bhn), (ind, t_ind), (wout, t_wout),
                           (bout, t_bout)]:
                nc.sync.dma_start(sb[:], dr[:])
            h = hpool.tile([128, NL, B], bf16)
            nc.any.memset(h[:], 0.0)

            # staging for diffusion hops, baseline layout [(w%8)*16+f, cc, nl, b]
            xc1 = spool.tile([128, 3, NL, B], bf16)
            xc2 = spool.tile([128, 3, NL, B], bf16)

            # ---- phase 1: diffusion hops (contract over source nodes) ----
            with (
                tc.tile_pool(name="xmb", bufs=2) as xmpool,
                tc.tile_pool(name="p1", bufs=2, space="PSUM") as p1pool,
            ):
                for b in range(B):
                    xmb = xmpool.tile([128, 4, WT * F], bf16)
                    nc.sync.dma_start(xmb[:], t_xm[b])
                    P1 = p1pool.tile([128, 3, NL], f32, tag="P1")
                    P2 = p1pool.tile([128, 3, NL], f32, tag="P2")
                    for mc in range(4):
                        for cc in range(3):
                            lhsT = xmb[:, mc, 128 * cc:128 * cc + 128]
                            nc.tensor.matmul(P1[:, cc, :], lhsT, a1t[:, mc, :],
                                             start=(mc == 0), stop=(mc == 3))
                            nc.tensor.matmul(P2[:, cc, :], lhsT, a2t[:, mc, :],
                                             start=(mc == 0), stop=(mc == 3))
                    nc.vector.tensor_copy(xc1[:, :, :, b], P1[:])
                    nc.scalar.copy(xc2[:, :, :, b], P2[:])

            # ---- phase 2: GRU over time, input projection fused into Wc ----
            with (
                tc.tile_pool(name="xs", bufs=2) as xspool,
                tc.tile_pool(name="gp", bufs=2, space="PSUM") as gppool,
                tc.tile_pool(name="hn", bufs=2, space="PSUM") as hnpool,
                tc.tile_pool(name="ew", bufs=2) as ewpool,
            ):
                for t in range(NT):
                    xs = xspool.tile([128, NL, 2, B], bf16, tag="xs")
                    if t < 2:
                        # rows 49-127 are contracted against zero weight rows;
                        # zero them once per ring buffer so no NaNs flow
                        nc.sync.dma_start(xs[KC:128, :, :, :], t_zpad[:])
                    # hop0 rows 0-15 + ones row 16 straight from DRAM
                    nc.sync.dma_start(xs[0:17, :, :, :], t_x0b[t])
                    # hop1/hop2 rows via SBUF->SBUF partition restructure
                    for wsub in range(2):
                        w = 2 * t + wsub
                        wo, ccw = w % 8, w // 8
                        src1 = xc1[16 * wo:16 * wo + 16, ccw, :, :]
                        src2 = xc2[16 * wo:16 * wo + 16, ccw, :, :]
                        nc.sync.dma_start(xs[17:33, :, wsub, :], src1)
                        nc.sync.dma_start(xs[33:49, :, wsub, :], src2)

                    for wsub in range(2):
                        for g in range(8):
                            ns = slice(8 * g, 8 * g + 8)
                            P = gppool.tile([128, 8, 3, B], f32, tag="P")
                            Phn = hnpool.tile([128, 8, B], f32, tag="Phn")
                            # b_hn via indicator matmul (opens accumulation)
                            nc.tensor.matmul(Phn[:], bhn[:, g, :], ind[:],
                                             start=True, stop=False,
                                             skip_group_check=True)
                            for j in range(8):
                                n = 8 * g + j
                                xsn = xs[:, n, wsub, :]
                                hn_ = h[:, n, :]
                                for gc in range(2):
                                    o = P[:, j, gc, :]
                                    nc.tensor.matmul(o, wc[:, n, gc, :], xsn,
                                                     start=True, stop=False)
                                    nc.tensor.matmul(o, whh[:, n, gc, :], hn_,
                                                     start=False, stop=True)
                                nc.tensor.matmul(P[:, j, 2, :], wc[:, n, 2, :],
                                                 xsn, start=True, stop=True)
                                nc.tensor.matmul(Phn[:, j, :], whh[:, n, 2, :],
                                                 hn_, start=False,
                                                 stop=(j == 7),
                                                 skip_group_check=True)
                            # ---- gate elementwise ----
                            rz = ewpool.tile([128, 8, 2, B], bf16, tag="rz")
                            nc.scalar.activation(rz[:], P[:, :, 0:2, :],
                                                 Act.Sigmoid)
                            tt = ewpool.tile([128, 8, B], bf16, tag="tt")
                            nc.vector.tensor_tensor(tt[:], rz[:, :, 0, :],
                                                    Phn[:], Alu.mult)
                            ut = ewpool.tile([128, 8, B], bf16, tag="ut")
                            nc.vector.tensor_tensor(ut[:], tt[:], P[:, :, 2, :],
                                                    Alu.add)
                            nt = ewpool.tile([128, 8, B], bf16, tag="nt")
                            nc.scalar.activation(nt[:], ut[:], Act.Tanh)
                            st = ewpool.tile([128, 8, B], bf16, tag="st")
                            nc.gpsimd.tensor_tensor(st[:], h[:, ns, :], nt[:],
                                                    Alu.subtract)
                            vt = ewpool.tile([128, 8, B], bf16, tag="vt")
                            nc.vector.tensor_tensor(vt[:], rz[:, :, 1, :], st[:],
                                                    Alu.mult)
                            nc.vector.tensor_tensor(h[:, ns, :], nt[:], vt[:],
                                                    Alu.add)

            # ---- output projection ----
            with (
                tc.tile_pool(name="po", bufs=1, space="PSUM") as popool,
                tc.tile_pool(name="ou", bufs=1) as oupool,
            ):
                Po = popool.tile([128, 32, F], f32)
                for c in range(32):
                    nc.tensor.matmul(Po[:, c, :], h[:, 2 * c:2 * c + 2, :],
                                     wout[:], start=True, stop=True)
                outsb = oupool.tile([128, 32, F], f32)
                nc.vector.tensor_tensor(
                    outsb[:], Po[:], bout[:, None, :].to_broadcast((128, 32, F)),
                    Alu.add)
                nc.sync.dma_start(t_out[:], outsb[:])
    nc.compile()


def kernel(**inputs):
    import concourse.bacc as bacc
    import concourse.bass as bass
    import concourse.mybir as mybir
    import concourse.tile as tile
    from concourse import bass_utils

    x = np.asarray(inputs["x"], np.float32)
    A = np.asarray(inputs["A_fw"], np.float32)
    dcw = np.asarray(inputs["dc_weights"], np.float32)
    W_ih = np.asarray(inputs["W_ih"], np.float32)
    W_hh = np.asarray(inputs["W_hh"], np.float32)
    b_ih = np.asarray(inputs["b_ih"], np.float32)
    b_hh = np.asarray(inputs["b_hh"], np.float32)
    W_out = np.asarray(inputs["W_out"], np.float32)
    b_out = np.asarray(inputs["b_out"], np.float32)

    A2 = A @ A
    dc_all = np.stack([dcw[0:16], dcw[16:32] + dcw[32:48], dcw[48:64] + dcw[64:80]])
    xbf = x.astype(BF)
    xm = np.ascontiguousarray(xbf.reshape(B, 4, 128, WT * F).transpose(0, 2, 1, 3))
    wout_h = W_out.astype(BF)
    bout_h = np.tile(b_out[None, :], (128, 1)).astype(np.float32)

    # fused input weights: Wc[n, gate, j, (hop, f)] = sum_h W_ih[n, gj, h] dc[hop, f, h]
    Wg = W_ih.reshape(N, 3, 128, H)
    wc_full = np.einsum('ngjh,ofh->ngjof', Wg, dc_all)        # [N, 3, 128, 3, 16]
    bias_in = b_ih.reshape(N, 3, 128).copy()
    bias_in[:, 0:2, :] += b_hh.reshape(N, 3, 128)[:, 0:2, :]  # r,z combined bias
    # K-layout: rows 0-15 hop0, 16 ones(bias), 17-32 hop1, 33-48 hop2,
    # 49-127 zero padding (keeps K=128 so fast weight load stays enabled)
    wc_k = np.zeros((N, 3, 128, 128), np.float32)
    wc_k[:, :, 0:16, :] = wc_full[:, :, :, 0, :].transpose(0, 1, 3, 2)
    wc_k[:, :, 16, :] = bias_in
    wc_k[:, :, 17:33, :] = wc_full[:, :, :, 1, :].transpose(0, 1, 3, 2)
    wc_k[:, :, 33:49, :] = wc_full[:, :, :, 2, :].transpose(0, 1, 3, 2)

    ind_h = np.zeros((128, 8, B), np.float32)
    for k in range(8):
        ind_h[k, k, :] = 1.0
    ind_h = ind_h.astype(BF)

    in_maps = []
    for c in range(8):
        ns = slice(c * NL, (c + 1) * NL)
        a1t = np.ascontiguousarray(
            A[ns].T.astype(BF).reshape(4, 128, NL).transpose(1, 0, 2))
        a2t = np.ascontiguousarray(
            A2[ns].T.astype(BF).reshape(4, 128, NL).transpose(1, 0, 2))
        xl = xbf[:, ns]  # [b, nl, w, f]
        # x0b[t, p, nl, wsub, b]: p 0-15 = f rows of x[w=2t+wsub], p16 = ones
        x0b = np.empty((NT, 17, NL, 2, B), np.float32)
        xw = np.asarray(xl, np.float32).transpose(2, 3, 1, 0)  # [w, f, nl, b]
        x0b[:, 0:16] = xw.reshape(NT, 2, 16, NL, B).transpose(0, 2, 3, 1, 4)
        x0b[:, 16] = 1.0
        wc_h = np.ascontiguousarray(
            wc_k[ns].transpose(2, 0, 1, 3)).astype(BF)       # [128, NL, 3, 128]
        whh_h = np.ascontiguousarray(
            W_hh[ns].transpose(2, 0, 1).astype(BF).reshape(128, NL, 3, 128))
        bhn_h = np.zeros((128, 8, 128), np.float32)           # [n8(+pad), g, j]
        bhn_h[0:8] = b_hh[ns, 256:384].reshape(8, 8, 128).transpose(1, 0, 2)
        bhn_h = bhn_h.astype(BF)
        in_maps.append({
            "a1t": a1t, "a2t": a2t, "xm": xm,
            "x0b": np.ascontiguousarray(x0b).astype(BF),
            "wc": wc_h, "whh": whh_h, "bhn": bhn_h, "ind": ind_h,
            "zpad": np.zeros((128 - KC, NL, 2, B), BF),
            "wout": wout_h, "bout": bout_h,
        })

    nc = bacc.Bacc("TRN2", target_bir_lowering=False, debug=False, num_devices=8)
    _build(nc, bass, mybir, tile)
    import os, time
    trace = bool(os.environ.get("DGCN_TRACE"))
    res = bass_utils.run_bass_kernel_spmd(nc, in_maps, core_ids=list(range(8)),
                                          trace=trace)
    if trace and res.exec_time_ns:
        print(f"MEASURED exec_time_ns: {res.exec_time_ns}", flush=True)
        try:
            with open("/tmp/dgcn_exec_ns.txt", "w") as f:
                f.write(str(res.exec_time_ns))
        except Exception:
            pass
        if res.instructions_and_trace:
            print(f"trace: {res.instructions_and_trace[1]}", flush=True)
    if os.environ.get("DGCN_BENCH"):
        for it in range(int(os.environ["DGCN_BENCH"])):
            t0 = time.time()
            res = bass_utils.run_bass_kernel_spmd(nc, in_maps, core_ids=list(range(8)))
            print(f"bench iter {it}: {(time.time()-t0)*1e3:.1f} ms", flush=True)

    out = np.zeros((B, N, F), np.float32)
    for c in range(8):
        arr = res.results[c]["out"]  # [128, 32, F]
        tmp = arr.transpose(1, 0, 2).reshape(32, 2, B, F).transpose(2, 0, 1, 3)
        out[:, c * NL:(c + 1) * NL] = tmp.reshape(B, NL, F)
    return out
